# revision 21
# baseline (speedup 1.0000x reference)
"""CartesianMACE rank-0 fused kernel for 8 trn2 NeuronCores (fp16 edition).

The reference's ranks 1 and 2 never reach the output (each rank is mixed
independently and the head reads only h[0]), so only the rank-0 slices of
cw0/mw0/cw1/mw1 plus h0/msg0_r0/msg1_r0/w_pred/b_pred are needed.

Per node n (16x16 mats A=cw0[0,n], B=mw0[0,n], D=cw1[0,n], E=mw1[0,n];
16-vecs x=h0[n], m0=msg0_r0[n], m1=msg1_r0[n]):
    s[n] = colsum(D) . (A x + B m0) + colsum(E) . m1
    out  = [sum_n s[n] w_pred[0,n], sum_n s[n] w_pred[1,n]] + b_pred

All tensors are downcast to fp16 on the host (harness tolerance is 2e-2;
fp16 lands ~1e-4), which halves HBM traffic AND doubles DVE throughput
(16-bit tensor_tensor runs in 2x_1P mode — HW-verified, including
broadcast operands and strided even-run folds). All reductions are binary
fold trees of tensor_add at 2x — tensor_reduce only has a 1x uop.

Engine split per chunk (7 chunks x 7 node-groups): DVE does products,
the A+B merge fold and all k-folds down to width 2, plus DE folds 8->4->2;
GpSimd does only the DE level-1 fold (16->8) plus the width-1 tail folds
(1x on DVE anyway), deferred two chunks so it never stalls on DVE. ab/de
stay fully SBUF-resident; DMA arrives in 4 big slices per tensor on the
two HWDGE rings (sync + scalar).

Sharding: data-parallel over nodes. 50000 nodes padded to 50176 =
8 cores x 128 partitions x 49 groups. Per-core [128, 2] f32 partial head
outputs are summed on host (the all-reduce of the head).
"""

import sys

for _p in ("/opt/trn_rl_repo", "/root/.axon_site/_ro/trn_rl_repo"):
    if _p not in sys.path:
        sys.path.append(_p)

import numpy as np

N, CH = 50000, 16
CORES = 8
P = 128
GPP = 49                  # node groups per partition
NP = CORES * P * GPP      # 50176 padded nodes
NCHUNK = 7                # processing chunks per core
G = GPP // NCHUNK         # groups per chunk
DMA_SLICES = (7, 14, 14, 7, 7)   # group counts per ab/de DMA slice

_cache = {}
TRACE = False


def _split_multiwait(nc, mybir):
    """This walrus build accepts a single sync-wait per instruction, but Tile
    attaches one wait per producer proc. Split: keep the last wait on the
    instruction and hoist the rest onto fresh same-engine Drain carriers
    inserted immediately before it (engines execute their stream in-order,
    so semantics are identical)."""
    for fn in nc.m.functions:
        for bb in fn.blocks:
            insts = bb.instructions  # live list
            i = 0
            while i < len(insts):
                ins = insts[i]
                si = ins.sync_info
                if si is not None and len(si.on_wait) > 1:
                    waits = list(si.on_wait)
                    ins.sync_info = mybir.SyncInfo(
                        on_wait=waits[-1:], on_update=list(si.on_update))
                    for k, w in enumerate(waits[:-1]):
                        insts.insert(i + k, mybir.InstDrain(
                            name=f"{ins.name}_w{k}", opcode="Drain",
                            engine=ins.engine, ins=[], outs=[],
                            sync_info=mybir.SyncInfo(on_wait=[w], on_update=[]),
                        ))
                    i += len(waits) - 1
                i += 1


def _unleash_input_dmas(nc, names):
    """Tile's scheduler ties input-DMA issue to compute progress (waits on
    DVE_*/Pool_* engine sems), serializing prefetch behind the chunk loop.
    Those waits are scheduling artifacts, not data deps — every reader
    waits on the DMA-completion sem itself, and the destination tiles are
    written exactly once. Strip engine-progress waits from the input loads
    (keep DMAHW* lane-reuse waits) so they stream at full rate."""
    for fn in nc.m.functions:
        for bb in fn.blocks:
            for ins in bb.instructions:
                if ins.name in names and ins.sync_info is not None:
                    si = ins.sync_info
                    upd = {u.ant_name for u in si.on_update}
                    keep = [w for w in si.on_wait if w.ant_name in upd]
                    ins.sync_info = type(si)(
                        on_wait=keep, on_update=list(si.on_update))


def _build_nc():
    import concourse.bass as bass
    import concourse.tile as tile
    import concourse.mybir as mybir

    f16 = mybir.dt.float16
    f32 = mybir.dt.float32
    ADD = mybir.AluOpType.add

    nc = bass.Bass("TRN2", target_bir_lowering=False, debug=False,
                   num_devices=CORES)

    # per-partition free layouts:
    #   ab: g, j(16), m(2), k(16)   de: g, q(2), j(16), i(16)
    #   xm: g, m(2), k(16)          m1: g, j(16)        w: c(2), g(GPP)
    ab_d = nc.dram_tensor("ab", [P, GPP * 512], f16, kind="ExternalInput").ap()
    de_d = nc.dram_tensor("de", [P, GPP * 512], f16, kind="ExternalInput").ap()
    xm_d = nc.dram_tensor("xm", [P, GPP * 32], f16, kind="ExternalInput").ap()
    m1_d = nc.dram_tensor("m1", [P, GPP * 16], f16, kind="ExternalInput").ap()
    w_d = nc.dram_tensor("w", [P, 2 * GPP], f16, kind="ExternalInput").ap()
    o_d = nc.dram_tensor("o", [P, 2], f32, kind="ExternalOutput").ap()

    with tile.TileContext(nc) as tc:
        with (
            tc.tile_pool(name="work", bufs=2) as work,
            tc.tile_pool(name="acc", bufs=1) as acc,
        ):
            # SBUF-resident inputs + accumulators. ab/de are split into one
            # tile per DMA slice — Tile dependency tracking is per-tile, so
            # a chunk's reads must not wait on unrelated slices' DMAs.
            xm_all = acc.tile([P, GPP * 32], f16)
            m1_sb = acc.tile([P, GPP * 16], f16)
            w_sb = acc.tile([P, 2 * GPP], f16)
            tab1 = acc.tile([P, GPP * 16], f16)   # (Ax+Bm0) per (g, j)
            dcol1 = acc.tile([P, GPP * 32], f16)  # colsums per (g, q, j)
            # All input loads on ONE ring (SP), in exact consumption order —
            # the two HWDGE rings do not share SDMA bandwidth fairly (the
            # SP ring effectively starves the ACT ring while it has work).
            in_dmas = []
            in_dmas.append(nc.sync.dma_start(out=xm_all[:, :], in_=xm_d))
            ab_t, de_t, sl_of = [], [], {}
            off = 0
            for si, sl in enumerate(DMA_SLICES):
                a = acc.tile([P, sl * 512], f16, tag=f"ab{si}")
                in_dmas.append(nc.sync.dma_start(
                    out=a[:, :], in_=ab_d[:, off * 512:(off + sl) * 512]))
                d = acc.tile([P, sl * 512], f16, tag=f"de{si}")
                in_dmas.append(nc.sync.dma_start(
                    out=d[:, :], in_=de_d[:, off * 512:(off + sl) * 512]))
                ab_t.append(a)
                de_t.append(d)
                for g in range(off // G, (off + sl) // G):
                    sl_of[g] = (si, off // G)
                off += sl
            in_dmas.append(nc.sync.dma_start(out=m1_sb[:, :], in_=m1_d))
            in_dmas.append(nc.sync.dma_start(out=w_sb[:, :], in_=w_d))

            # persistent intermediates consumed out-of-chunk
            d8_all = acc.tile([P, GPP * 256], f16)
            tw_all = acc.tile([P, GPP * 32], f16)
            d2_all = acc.tile([P, GPP * 64], f16)

            # gpsimd pass 1: all DE level-1 folds (i 16->8), DMA-paced only
            for c in range(NCHUNK):
                si, base = sl_of[c]
                lo = (c - base) * G * 512
                de5 = de_t[si][:, lo:lo + G * 512].rearrange(
                    "p (g q j i) -> p g q j i", g=G, q=2, j=16, i=16)
                d8r = d8_all[:, c * G * 256:(c + 1) * G * 256].rearrange(
                    "p (g q j i) -> p g q j i", g=G, q=2, j=16, i=8)
                nc.gpsimd.tensor_add(out=d8r, in0=de5[:, :, :, :, 0:8],
                                     in1=de5[:, :, :, :, 8:16])

            # vector pass: per-chunk products + folds
            for c in range(NCHUNK):
                si, base = sl_of[c]
                lo = (c - base) * G * 512
                ab_sb = ab_t[si][:, lo:lo + G * 512]
                d8r = d8_all[:, c * G * 256:(c + 1) * G * 256].rearrange(
                    "p (g q j i) -> p g q j i", g=G, q=2, j=16, i=8)

                # products P[g, j, m, k] = AB * xm (xm bcast over j)
                pt = work.tile([P, G * 512], f16, tag="pt")
                p5 = pt[:, :].rearrange("p (g j m k) -> p g j m k",
                                        g=G, j=16, m=2, k=16)
                ab5 = ab_sb.rearrange("p (g j m k) -> p g j m k",
                                      g=G, j=16, m=2, k=16)
                xm_bc = (xm_all[:, c * G * 32:(c + 1) * G * 32]
                         .rearrange("p (g m k) -> p g m k", g=G, m=2, k=16)
                         .unsqueeze(2).broadcast_to((P, G, 16, 2, 16)))
                nc.vector.tensor_mul(out=p5, in0=ab5, in1=xm_bc)

                # m-fold: t1[g, j, k] = P[..., A, k] + P[..., B, k]
                t1 = work.tile([P, G * 256], f16, tag="t1")
                t1r = t1[:, :].rearrange("p (g j k) -> p g j k",
                                         g=G, j=16, k=16)
                nc.vector.tensor_add(out=t1r, in0=p5[:, :, :, 0],
                                     in1=p5[:, :, :, 1])
                # k-folds 16 -> 2
                t2 = work.tile([P, G * 128], f16, tag="t2")
                t2r = t2[:, :].rearrange("p (g j k) -> p g j k",
                                         g=G, j=16, k=8)
                nc.vector.tensor_add(out=t2r, in0=t1r[:, :, :, 0:8],
                                     in1=t1r[:, :, :, 8:16])
                t4 = work.tile([P, G * 64], f16, tag="t4")
                t4r = t4[:, :].rearrange("p (g j k) -> p g j k",
                                         g=G, j=16, k=4)
                nc.vector.tensor_add(out=t4r, in0=t2r[:, :, :, 0:4],
                                     in1=t2r[:, :, :, 4:8])
                twr = tw_all[:, c * G * 32:(c + 1) * G * 32].rearrange(
                    "p (g j k) -> p g j k", g=G, j=16, k=2)
                nc.vector.tensor_add(out=twr, in0=t4r[:, :, :, 0:2],
                                     in1=t4r[:, :, :, 2:4])

                # DE folds L2/L3 on vector (8 -> 4 -> 2)
                d4 = work.tile([P, G * 128], f16, tag="d4")
                d4r = d4[:, :].rearrange("p (g q j i) -> p g q j i",
                                         g=G, q=2, j=16, i=4)
                nc.vector.tensor_add(out=d4r, in0=d8r[:, :, :, :, 0:4],
                                     in1=d8r[:, :, :, :, 4:8])
                d2r = d2_all[:, c * G * 64:(c + 1) * G * 64].rearrange(
                    "p (g q j i) -> p g q j i", g=G, q=2, j=16, i=2)
                nc.vector.tensor_add(out=d2r, in0=d4r[:, :, :, :, 0:2],
                                     in1=d4r[:, :, :, :, 2:4])

            # gpsimd pass 2: width-1 tail folds (1x on DVE anyway)
            twa = tw_all[:, :].rearrange("p (g j k) -> p g j k",
                                         g=GPP, j=16, k=2)
            d2a = d2_all[:, :].rearrange("p (g q j i) -> p g q j i",
                                         g=GPP, q=2, j=16, i=2)
            for c in range(NCHUNK):
                ot = c * G * 16
                tslice = tab1[:, ot:ot + G * 16].rearrange(
                    "p (g j) -> p g j", g=G, j=16)
                nc.gpsimd.tensor_add(
                    out=tslice,
                    in0=twa[:, c * G:(c + 1) * G, :, 0],
                    in1=twa[:, c * G:(c + 1) * G, :, 1])
                od = c * G * 32
                dslice = dcol1[:, od:od + G * 32].rearrange(
                    "p (g q j) -> p g q j", g=G, q=2, j=16)
                nc.gpsimd.tensor_add(
                    out=dslice,
                    in0=d2a[:, c * G:(c + 1) * G, :, :, 0],
                    in1=d2a[:, c * G:(c + 1) * G, :, :, 1])

            # ---- epilogue ----
            # R[m, g, j]: m=0 -> tab1 * dcolD, m=1 -> m1 * dcolE
            dv = dcol1[:, :].rearrange("p (g q j) -> p g q j",
                                       g=GPP, q=2, j=16)
            r = acc.tile([P, 2 * GPP * 16], f16)
            rv = r[:, :].rearrange("p (m g j) -> p m g j",
                                   m=2, g=GPP, j=16)
            nc.vector.tensor_mul(
                out=rv[:, 0],
                in0=tab1[:, :].rearrange("p (g j) -> p g j", g=GPP, j=16),
                in1=dv[:, :, 0])
            nc.vector.tensor_mul(
                out=rv[:, 1],
                in0=m1_sb[:, :].rearrange("p (g j) -> p g j", g=GPP, j=16),
                in1=dv[:, :, 1])
            # fold m then j: 16 -> 8 -> 4 -> 2 -> 1
            sm = acc.tile([P, GPP * 16], f16)
            nc.vector.tensor_add(out=sm[:, :], in0=r[:, 0:GPP * 16],
                                 in1=r[:, GPP * 16:2 * GPP * 16])
            smr = sm[:, :].rearrange("p (g j) -> p g j", g=GPP, j=16)
            s8 = acc.tile([P, GPP * 8], f16)
            s8r = s8[:, :].rearrange("p (g j) -> p g j", g=GPP, j=8)
            nc.vector.tensor_add(out=s8r, in0=smr[:, :, 0:8],
                                 in1=smr[:, :, 8:16])
            s4 = acc.tile([P, GPP * 4], f16)
            s4r = s4[:, :].rearrange("p (g j) -> p g j", g=GPP, j=4)
            nc.vector.tensor_add(out=s4r, in0=s8r[:, :, 0:4],
                                 in1=s8r[:, :, 4:8])
            s2 = acc.tile([P, GPP * 2], f16)
            s2r = s2[:, :].rearrange("p (g j) -> p g j", g=GPP, j=2)
            nc.vector.tensor_add(out=s2r, in0=s4r[:, :, 0:2],
                                 in1=s4r[:, :, 2:4])
            s1 = acc.tile([P, GPP], f16)
            nc.vector.tensor_add(
                out=s1[:, :].rearrange("p g -> p g"),
                in0=s2r[:, :, 0], in1=s2r[:, :, 1])

            # head: o[:, c] = sum_g s1[:, g] * w[:, c, g]  (f32 accumulate)
            hp = acc.tile([P, 2 * GPP], f16)
            hpv = hp[:, :].rearrange("p (c g) -> p c g", c=2, g=GPP)
            nc.vector.tensor_mul(
                out=hpv,
                in0=w_sb[:, :].rearrange("p (c g) -> p c g", c=2, g=GPP),
                in1=s1[:, :].rearrange("p g -> p g").unsqueeze(1)
                .broadcast_to((P, 2, GPP)))
            o_sb = acc.tile([P, 2], f32)
            nc.vector.tensor_reduce(
                out=o_sb[:, :].rearrange("p c -> p c"),
                in_=hpv, axis=mybir.AxisListType.X, op=ADD)
            nc.sync.dma_start(out=o_d, in_=o_sb[:, :])

    nc._input_dma_names = {i.ins.name for i in in_dmas}
    return nc


def _get_nc():
    if "nc" not in _cache:
        _cache["nc"] = _build_nc()
    return _cache["nc"]


def _shard(x):
    """(N, ...) f32 -> (CORES, 128, GPP, ...) fp16, zero padded.
    Node mapping: n = (core*128 + p)*GPP + g."""
    out = np.zeros((NP,) + x.shape[1:], np.float16)
    out[:N] = x.astype(np.float16)
    return out.reshape((CORES, P, GPP) + x.shape[1:])


def kernel(h0, cw0, mw0, cw1, mw1,
           msg0_r0, msg0_r1, msg0_r2,
           msg1_r0, msg1_r1, msg1_r2,
           w_pred, b_pred):
    from concourse.bass_utils import run_bass_kernel_spmd

    nc = _get_nc()
    if not _cache.get("split_done"):
        import concourse.mybir as mybir
        _unleash_input_dmas(nc, nc._input_dma_names)
        _split_multiwait(nc, mybir)
        _cache["split_done"] = True

    A = np.asarray(cw0[0], np.float32)
    B = np.asarray(mw0[0], np.float32)
    D = np.asarray(cw1[0], np.float32)
    E = np.asarray(mw1[0], np.float32)

    # ab[n, j, m, k] = {A,B}[n, j, k]
    AB = _shard(np.stack([A, B], axis=2)).reshape(CORES, P, GPP * 512)
    # de[n, q, j, i] = {D,E}[n, i, j]  (reduce dim i innermost)
    DE = _shard(np.stack([D.transpose(0, 2, 1), E.transpose(0, 2, 1)],
                         axis=1)).reshape(CORES, P, GPP * 512)
    # xm[n, m, k] = {x, m0}[n, k]
    XM = _shard(np.stack([np.asarray(h0, np.float32)[..., 0],
                          np.asarray(msg0_r0, np.float32)[..., 0]],
                         axis=1)).reshape(CORES, P, GPP * 32)
    M1 = _shard(np.asarray(msg1_r0, np.float32)[..., 0]
                ).reshape(CORES, P, GPP * 16)

    wp = np.zeros((2, NP), np.float32)
    wp[:, :N] = np.asarray(w_pred, np.float32)
    W = np.ascontiguousarray(
        wp.reshape(2, CORES, P, GPP).transpose(1, 2, 0, 3)
        .reshape(CORES, P, 2 * GPP)).astype(np.float16)

    in_maps = [
        {"ab": np.ascontiguousarray(AB[i]),
         "de": np.ascontiguousarray(DE[i]),
         "xm": np.ascontiguousarray(XM[i]),
         "m1": np.ascontiguousarray(M1[i]),
         "w": np.ascontiguousarray(W[i])}
        for i in range(CORES)
    ]
    res = run_bass_kernel_spmd(nc, in_maps, list(range(CORES)), trace=TRACE)
    _cache["last_res"] = res
    partial = np.zeros(2, np.float64)
    for i in range(CORES):
        partial += res.results[i]["o"].astype(np.float64).sum(axis=0)
    out = (partial + np.asarray(b_pred, np.float64)).astype(np.float32)
    return out.reshape(1, 2)


# revision 29
# speedup vs baseline: 1.1911x; 1.1911x over previous
"""CartesianMACE rank-0 fused kernel for 8 trn2 NeuronCores (fp16, all-DVE).

The reference's ranks 1 and 2 never reach the output (each rank is mixed
independently and the head reads only h[0]), so only the rank-0 slices of
cw0/mw0/cw1/mw1 plus h0/msg0_r0/msg1_r0/w_pred/b_pred are needed.

Per node n (16x16 mats A=cw0[0,n], B=mw0[0,n], D=cw1[0,n], E=mw1[0,n];
16-vecs x=h0[n], m0=msg0_r0[n], m1=msg1_r0[n]):
    s[n] = colsum(D) . (A x + B m0) + colsum(E) . m1
    out  = [sum_n s[n] w_pred[0,n], sum_n s[n] w_pred[1,n]] + b_pred

Design notes (HW-measured on this device):
- fp16 halves HBM bytes and doubles DVE tensor_tensor throughput (2x_1P
  engages for contiguous, strided-even-run, and broadcast operands).
- All reductions are binary fold trees of tensor_add at 2x; tensor_reduce
  and pool only have 1x uops (pool doesn't even encode on this walrus).
- GPSIMD shares its SBUF port with the DVE ("POOL slot"): concurrent
  gpsimd work slows DVE ops up to 4-6x, so gpsimd is net-negative for this
  DVE-bound kernel — everything runs on the DVE.
- The two HWDGE rings share SDMA engines unfairly (SP starves ACT), so all
  latency-critical loads go on the SP ring in consumption order; only the
  epilogue-only m1/w go on ACT. ab/de are interleaved chunk-wise in one
  dram tensor so each chunk arrives as one large DMA.
- Tile's scheduler ties input-DMA issue to compute progress; those waits
  are stripped post-hoc (_unleash_input_dmas) — destinations are
  write-once tiles and readers wait on the DMA-completion sems.

Sharding: data-parallel over nodes. 50000 nodes padded to 50176 =
8 cores x 128 partitions x 49 groups. Per-core [128, 2] f32 partial head
outputs are summed on host (the all-reduce of the head).
"""

import sys

for _p in ("/opt/trn_rl_repo", "/root/.axon_site/_ro/trn_rl_repo"):
    if _p not in sys.path:
        sys.path.append(_p)

import numpy as np

N, CH = 50000, 16
CORES = 8
P = 128
GPP = 49                  # node groups per partition
NP = CORES * P * GPP      # 50176 padded nodes
CHUNKS = (7, 14, 14, 7, 7)   # node groups per compute chunk / DMA slice

_cache = {}
TRACE = False


def _split_multiwait(nc, mybir):
    """This walrus build accepts a single sync-wait per instruction, but Tile
    attaches one wait per producer proc. Split: keep the last wait on the
    instruction and hoist the rest onto fresh same-engine Drain carriers
    inserted immediately before it (engines execute their stream in-order,
    so semantics are identical)."""
    for fn in nc.m.functions:
        for bb in fn.blocks:
            insts = bb.instructions  # live list
            i = 0
            while i < len(insts):
                ins = insts[i]
                si = ins.sync_info
                if si is not None and len(si.on_wait) > 1:
                    waits = list(si.on_wait)
                    ins.sync_info = mybir.SyncInfo(
                        on_wait=waits[-1:], on_update=list(si.on_update))
                    for k, w in enumerate(waits[:-1]):
                        insts.insert(i + k, mybir.InstDrain(
                            name=f"{ins.name}_w{k}", opcode="Drain",
                            engine=ins.engine, ins=[], outs=[],
                            sync_info=mybir.SyncInfo(on_wait=[w], on_update=[]),
                        ))
                    i += len(waits) - 1
                i += 1


def _unleash_input_dmas(nc, names):
    """Strip scheduling-artifact waits from the input loads: keep only
    same-lane sem-reuse waits (a wait on the sem the DMA itself updates,
    which orders same-lane completions); engine-progress and cross-lane
    chain waits only serialize prefetch. Destinations are write-once tiles
    and every reader waits on the DMA-completion sems, so this is safe."""
    for fn in nc.m.functions:
        for bb in fn.blocks:
            for ins in bb.instructions:
                if ins.name in names and ins.sync_info is not None:
                    si = ins.sync_info
                    upd = {u.ant_name for u in si.on_update}
                    keep = [w for w in si.on_wait if w.ant_name in upd]
                    ins.sync_info = type(si)(
                        on_wait=keep, on_update=list(si.on_update))


def _build_nc():
    import concourse.bass as bass
    import concourse.tile as tile
    import concourse.mybir as mybir

    f16 = mybir.dt.float16
    f32 = mybir.dt.float32
    ADD = mybir.AluOpType.add

    nc = bass.Bass("TRN2", target_bir_lowering=False, debug=False,
                   num_devices=CORES)
    nc._opmap = {}

    def _lab(r, name):
        try:
            nc._opmap[r.ins.name] = name
        except Exception:
            pass
        return r

    # per-partition free layouts:
    #   abde: per chunk [ab-chunk | de-chunk], ab: g, j(16), m(2), k(16);
    #         de: g, q(2), j(16), i(16)
    #   xm: g, m(2), k(16)          m1: g, j(16)        w: c(2), g(GPP)
    abde_d = nc.dram_tensor("abde", [P, 2 * GPP * 512], f16,
                            kind="ExternalInput").ap()
    xm_d = nc.dram_tensor("xm", [P, GPP * 32], f16, kind="ExternalInput").ap()
    m1_d = nc.dram_tensor("m1", [P, GPP * 16], f16, kind="ExternalInput").ap()
    w_d = nc.dram_tensor("w", [P, 2 * GPP], f16, kind="ExternalInput").ap()
    o_d = nc.dram_tensor("o", [P, 2], f32, kind="ExternalOutput").ap()

    with tile.TileContext(nc) as tc:
        with (
            tc.tile_pool(name="work", bufs=1) as work,
            tc.tile_pool(name="acc", bufs=1) as acc,
        ):
            xm_all = acc.tile([P, GPP * 32], f16)
            m1_sb = acc.tile([P, GPP * 16], f16)
            w_sb = acc.tile([P, 2 * GPP], f16)
            tab1 = acc.tile([P, GPP * 16], f16)   # (Ax+Bm0) per (g, j)
            dcol1 = acc.tile([P, GPP * 32], f16)  # colsums per (g, q, j)

            in_dmas = []
            in_dmas.append(nc.sync.dma_start(out=xm_all[:, :], in_=xm_d))
            ch_t = []
            off = 0
            for ci, g in enumerate(CHUNKS):
                t = acc.tile([P, g * 1024], f16, tag=f"c{ci}")
                in_dmas.append(nc.sync.dma_start(
                    out=t[:, :],
                    in_=abde_d[:, off * 1024:(off + g) * 1024]))
                ch_t.append((t, off, g))
                off += g
            # epilogue-only data rides the (starved) ACT ring
            in_dmas.append(nc.scalar.dma_start(out=m1_sb[:, :], in_=m1_d))
            in_dmas.append(nc.scalar.dma_start(out=w_sb[:, :], in_=w_d))

            for ci, (ct, off, g) in enumerate(ch_t):
                ab_sb = ct[:, 0:g * 512]
                de_sb = ct[:, g * 512:g * 1024]

                # products P[g, j, m, k] = AB * xm (xm bcast over j)
                pt = work.tile([P, 14 * 512], f16, tag="pt")
                p5 = pt[:, 0:g * 512].rearrange(
                    "p (g j m k) -> p g j m k", g=g, j=16, m=2, k=16)
                ab5 = ab_sb.rearrange("p (g j m k) -> p g j m k",
                                      g=g, j=16, m=2, k=16)
                xm_bc = (xm_all[:, off * 32:(off + g) * 32]
                         .rearrange("p (g m k) -> p g m k", g=g, m=2, k=16)
                         .unsqueeze(2).broadcast_to((P, g, 16, 2, 16)))
                _lab(nc.vector.tensor_mul(out=p5, in0=ab5, in1=xm_bc),
                     f'prod{ci}')

                # m-fold then k-folds 16 -> 1
                t1 = work.tile([P, 14 * 256], f16, tag="t1")
                t1r = t1[:, 0:g * 256].rearrange(
                    "p (g j k) -> p g j k", g=g, j=16, k=16)
                _lab(nc.vector.tensor_add(out=t1r, in0=p5[:, :, :, 0],
                                          in1=p5[:, :, :, 1]), f'mfold{ci}')
                t2 = work.tile([P, 14 * 128], f16, tag="t2")
                t2r = t2[:, 0:g * 128].rearrange(
                    "p (g j k) -> p g j k", g=g, j=16, k=8)
                _lab(nc.vector.tensor_add(out=t2r, in0=t1r[:, :, :, 0:8],
                                          in1=t1r[:, :, :, 8:16]), f'kL2_{ci}')
                t4 = work.tile([P, 14 * 64], f16, tag="t4")
                t4r = t4[:, 0:g * 64].rearrange(
                    "p (g j k) -> p g j k", g=g, j=16, k=4)
                _lab(nc.vector.tensor_add(out=t4r, in0=t2r[:, :, :, 0:4],
                                          in1=t2r[:, :, :, 4:8]), f'kL3_{ci}')
                tw = work.tile([P, 14 * 32], f16, tag="tw")
                twr = tw[:, 0:g * 32].rearrange(
                    "p (g j k) -> p g j k", g=g, j=16, k=2)
                _lab(nc.vector.tensor_add(out=twr, in0=t4r[:, :, :, 0:2],
                                          in1=t4r[:, :, :, 2:4]), f'kL4_{ci}')
                tab_s = tab1[:, off * 16:(off + g) * 16].rearrange(
                    "p (g j) -> p g j", g=g, j=16)
                _lab(nc.vector.tensor_add(out=tab_s, in0=twr[:, :, :, 0],
                                          in1=twr[:, :, :, 1]), f'kL5_{ci}')

                # DE reduce (i 16 -> 1)
                de5 = de_sb.rearrange("p (g q j i) -> p g q j i",
                                      g=g, q=2, j=16, i=16)
                d8 = work.tile([P, 14 * 256], f16, tag="d8")
                d8r = d8[:, 0:g * 256].rearrange(
                    "p (g q j i) -> p g q j i", g=g, q=2, j=16, i=8)
                _lab(nc.vector.tensor_add(out=d8r, in0=de5[:, :, :, :, 0:8],
                                          in1=de5[:, :, :, :, 8:16]),
                     f'deL1_{ci}')
                d4 = work.tile([P, 14 * 128], f16, tag="d4")
                d4r = d4[:, 0:g * 128].rearrange(
                    "p (g q j i) -> p g q j i", g=g, q=2, j=16, i=4)
                _lab(nc.vector.tensor_add(out=d4r, in0=d8r[:, :, :, :, 0:4],
                                          in1=d8r[:, :, :, :, 4:8]),
                     f'd4_{ci}')
                d2 = work.tile([P, 14 * 64], f16, tag="d2")
                d2r = d2[:, 0:g * 64].rearrange(
                    "p (g q j i) -> p g q j i", g=g, q=2, j=16, i=2)
                _lab(nc.vector.tensor_add(out=d2r, in0=d4r[:, :, :, :, 0:2],
                                          in1=d4r[:, :, :, :, 2:4]),
                     f'd2_{ci}')
                dc_s = dcol1[:, off * 32:(off + g) * 32].rearrange(
                    "p (g q j) -> p g q j", g=g, q=2, j=16)
                _lab(nc.vector.tensor_add(out=dc_s, in0=d2r[:, :, :, :, 0],
                                          in1=d2r[:, :, :, :, 1]),
                     f'deL4_{ci}')

            # ---- epilogue ----
            # R[m, g, j]: m=0 -> tab1 * dcolD, m=1 -> m1 * dcolE
            dv = dcol1[:, :].rearrange("p (g q j) -> p g q j",
                                       g=GPP, q=2, j=16)
            r = acc.tile([P, 2 * GPP * 16], f16)
            rv = r[:, :].rearrange("p (m g j) -> p m g j",
                                   m=2, g=GPP, j=16)
            _lab(nc.vector.tensor_mul(
                out=rv[:, 0],
                in0=tab1[:, :].rearrange("p (g j) -> p g j", g=GPP, j=16),
                in1=dv[:, :, 0]), 'R0')
            _lab(nc.vector.tensor_mul(
                out=rv[:, 1],
                in0=m1_sb[:, :].rearrange("p (g j) -> p g j", g=GPP, j=16),
                in1=dv[:, :, 1]), 'R1')
            # fold m then j: 16 -> 8 -> 4 -> 2 -> 1
            sm = acc.tile([P, GPP * 16], f16)
            nc.vector.tensor_add(out=sm[:, :], in0=r[:, 0:GPP * 16],
                                 in1=r[:, GPP * 16:2 * GPP * 16])
            smr = sm[:, :].rearrange("p (g j) -> p g j", g=GPP, j=16)
            s8 = acc.tile([P, GPP * 8], f16)
            s8r = s8[:, :].rearrange("p (g j) -> p g j", g=GPP, j=8)
            nc.vector.tensor_add(out=s8r, in0=smr[:, :, 0:8],
                                 in1=smr[:, :, 8:16])
            s4 = acc.tile([P, GPP * 4], f16)
            s4r = s4[:, :].rearrange("p (g j) -> p g j", g=GPP, j=4)
            nc.vector.tensor_add(out=s4r, in0=s8r[:, :, 0:4],
                                 in1=s8r[:, :, 4:8])
            s2 = acc.tile([P, GPP * 2], f16)
            s2r = s2[:, :].rearrange("p (g j) -> p g j", g=GPP, j=2)
            nc.vector.tensor_add(out=s2r, in0=s4r[:, :, 0:2],
                                 in1=s4r[:, :, 2:4])
            s1 = acc.tile([P, GPP], f16)
            nc.vector.tensor_add(
                out=s1[:, :].rearrange("p g -> p g"),
                in0=s2r[:, :, 0], in1=s2r[:, :, 1])

            # head: o[:, c] = sum_g s1[:, g] * w[:, c, g]  (f32 accumulate)
            hp = acc.tile([P, 2 * GPP], f16)
            hpv = hp[:, :].rearrange("p (c g) -> p c g", c=2, g=GPP)
            nc.vector.tensor_mul(
                out=hpv,
                in0=w_sb[:, :].rearrange("p (c g) -> p c g", c=2, g=GPP),
                in1=s1[:, :].rearrange("p g -> p g").unsqueeze(1)
                .broadcast_to((P, 2, GPP)))
            o_sb = acc.tile([P, 2], f32)
            nc.vector.tensor_reduce(
                out=o_sb[:, :].rearrange("p c -> p c"),
                in_=hpv, axis=mybir.AxisListType.X, op=ADD)
            nc.sync.dma_start(out=o_d, in_=o_sb[:, :])

    nc._input_dma_names = {i.ins.name for i in in_dmas}
    return nc


def _get_nc():
    if "nc" not in _cache:
        _cache["nc"] = _build_nc()
    return _cache["nc"]


def _shard(x):
    """(N, ...) f32 -> (CORES, 128, GPP, ...) fp16, zero padded.
    Node mapping: n = (core*128 + p)*GPP + g."""
    out = np.zeros((NP,) + x.shape[1:], np.float16)
    out[:N] = x.astype(np.float16)
    return out.reshape((CORES, P, GPP) + x.shape[1:])


def kernel(h0, cw0, mw0, cw1, mw1,
           msg0_r0, msg0_r1, msg0_r2,
           msg1_r0, msg1_r1, msg1_r2,
           w_pred, b_pred):
    from concourse.bass_utils import run_bass_kernel_spmd

    nc = _get_nc()
    if not _cache.get("split_done"):
        import concourse.mybir as mybir
        _unleash_input_dmas(nc, nc._input_dma_names)
        _split_multiwait(nc, mybir)
        _cache["split_done"] = True

    A = np.asarray(cw0[0], np.float32)
    B = np.asarray(mw0[0], np.float32)
    D = np.asarray(cw1[0], np.float32)
    E = np.asarray(mw1[0], np.float32)

    # ab[n, j, m, k] = {A,B}[n, j, k]; de[n, q, j, i] = {D,E}[n, i, j]
    AB = _shard(np.stack([A, B], axis=2)).reshape(CORES, P, GPP, 512)
    DE = _shard(np.stack([D.transpose(0, 2, 1), E.transpose(0, 2, 1)],
                         axis=1)).reshape(CORES, P, GPP, 512)
    # interleave per DMA chunk: [ab-chunk | de-chunk]
    ABDE = np.empty((CORES, P, 2 * GPP * 512), np.float16)
    off = 0
    for g in CHUNKS:
        o2 = off * 1024
        ABDE[:, :, o2:o2 + g * 512] = AB[:, :, off:off + g].reshape(
            CORES, P, g * 512)
        ABDE[:, :, o2 + g * 512:o2 + g * 1024] = DE[:, :, off:off + g].reshape(
            CORES, P, g * 512)
        off += g

    XM = _shard(np.stack([np.asarray(h0, np.float32)[..., 0],
                          np.asarray(msg0_r0, np.float32)[..., 0]],
                         axis=1)).reshape(CORES, P, GPP * 32)
    M1 = _shard(np.asarray(msg1_r0, np.float32)[..., 0]
                ).reshape(CORES, P, GPP * 16)

    wp = np.zeros((2, NP), np.float32)
    wp[:, :N] = np.asarray(w_pred, np.float32)
    W = np.ascontiguousarray(
        wp.reshape(2, CORES, P, GPP).transpose(1, 2, 0, 3)
        .reshape(CORES, P, 2 * GPP)).astype(np.float16)

    in_maps = [
        {"abde": ABDE[i],
         "xm": np.ascontiguousarray(XM[i]),
         "m1": np.ascontiguousarray(M1[i]),
         "w": np.ascontiguousarray(W[i])}
        for i in range(CORES)
    ]
    res = run_bass_kernel_spmd(nc, in_maps, list(range(CORES)), trace=TRACE)
    _cache["last_res"] = res
    partial = np.zeros(2, np.float64)
    for i in range(CORES):
        partial += res.results[i]["o"].astype(np.float64).sum(axis=0)
    out = (partial + np.asarray(b_pred, np.float64)).astype(np.float32)
    return out.reshape(1, 2)


# revision 34
# speedup vs baseline: 1.2636x; 1.0609x over previous
"""CartesianMACE rank-0 fused kernel for 8 trn2 NeuronCores (fp16, all-DVE).

The reference's ranks 1 and 2 never reach the output (each rank is mixed
independently and the head reads only h[0]), so only the rank-0 slices of
cw0/mw0/cw1/mw1 plus h0/msg0_r0/msg1_r0/w_pred/b_pred are needed.

Per node n (16x16 mats A=cw0[0,n], B=mw0[0,n], D=cw1[0,n], E=mw1[0,n];
16-vecs x=h0[n], m0=msg0_r0[n], m1=msg1_r0[n]):
    s[n] = colsum(D) . (A x + B m0) + colsum(E) . m1
    out  = [sum_n s[n] w_pred[0,n], sum_n s[n] w_pred[1,n]] + b_pred

Design notes (HW-measured on this device):
- fp16 halves HBM bytes and doubles DVE tensor_tensor throughput (2x_1P
  engages for contiguous, strided-even-run, and broadcast operands).
- All reductions are binary fold trees of tensor_add at 2x; tensor_reduce
  and pool only have 1x uops (pool doesn't even encode on this walrus).
- GPSIMD shares its SBUF port with the DVE ("POOL slot"): concurrent
  gpsimd work slows DVE ops up to 4-6x, so gpsimd is net-negative for this
  DVE-bound kernel — everything runs on the DVE.
- The two HWDGE rings share SDMA engines unfairly (SP starves ACT), so all
  latency-critical loads go on the SP ring in consumption order; only the
  epilogue-only m1/w go on ACT. ab/de are interleaved chunk-wise in one
  dram tensor so each chunk arrives as one large DMA.
- Tile's scheduler ties input-DMA issue to compute progress; those waits
  are stripped post-hoc (_unleash_input_dmas) — destinations are
  write-once tiles and readers wait on the DMA-completion sems.

Sharding: data-parallel over nodes. 50000 nodes padded to 50176 =
8 cores x 128 partitions x 49 groups. Per-core [128, 2] f32 partial head
outputs are summed on host (the all-reduce of the head).
"""

import sys

for _p in ("/opt/trn_rl_repo", "/root/.axon_site/_ro/trn_rl_repo"):
    if _p not in sys.path:
        sys.path.append(_p)

import numpy as np

N, CH = 50000, 16
CORES = 8
P = 128
GPP = 49                  # node groups per partition
NP = CORES * P * GPP      # 50176 padded nodes
CHUNKS = (7, 14, 14, 7, 7)   # node groups per compute chunk / DMA slice

_cache = {}
TRACE = False


def _split_multiwait(nc, mybir):
    """This walrus build accepts a single sync-wait per instruction, but Tile
    attaches one wait per producer proc. Split: keep the last wait on the
    instruction and hoist the rest onto fresh same-engine Drain carriers
    inserted immediately before it (engines execute their stream in-order,
    so semantics are identical)."""
    for fn in nc.m.functions:
        for bb in fn.blocks:
            insts = bb.instructions  # live list
            i = 0
            while i < len(insts):
                ins = insts[i]
                si = ins.sync_info
                if si is not None and len(si.on_wait) > 1:
                    waits = list(si.on_wait)
                    ins.sync_info = mybir.SyncInfo(
                        on_wait=waits[-1:], on_update=list(si.on_update))
                    for k, w in enumerate(waits[:-1]):
                        insts.insert(i + k, mybir.InstDrain(
                            name=f"{ins.name}_w{k}", opcode="Drain",
                            engine=ins.engine, ins=[], outs=[],
                            sync_info=mybir.SyncInfo(on_wait=[w], on_update=[]),
                        ))
                    i += len(waits) - 1
                i += 1


def _unleash_input_dmas(nc, names):
    """Strip scheduling-artifact waits from the input loads: keep only
    same-lane sem-reuse waits (a wait on the sem the DMA itself updates,
    which orders same-lane completions); engine-progress and cross-lane
    chain waits only serialize prefetch. Destinations are write-once tiles
    and every reader waits on the DMA-completion sems, so this is safe."""
    for fn in nc.m.functions:
        for bb in fn.blocks:
            for ins in bb.instructions:
                if ins.name in names and ins.sync_info is not None:
                    si = ins.sync_info
                    upd = {u.ant_name for u in si.on_update}
                    keep = [w for w in si.on_wait if w.ant_name in upd]
                    ins.sync_info = type(si)(
                        on_wait=keep, on_update=list(si.on_update))


def _build_nc():
    import concourse.bass as bass
    import concourse.tile as tile
    import concourse.mybir as mybir

    f16 = mybir.dt.float16
    f32 = mybir.dt.float32
    ADD = mybir.AluOpType.add

    nc = bass.Bass("TRN2", target_bir_lowering=False, debug=False,
                   num_devices=CORES)
    nc._opmap = {}

    def _lab(r, name):
        try:
            nc._opmap[r.ins.name] = name
        except Exception:
            pass
        return r

    # per-partition free layouts:
    #   ab: g, j(16), m(2), k(16)   de: g, q(2), j(16), i(16)
    #   xm: g, m(2), k(16)          m1: g, j(16)        w: c(2), g(GPP)
    ab_d = nc.dram_tensor("ab", [P, GPP * 512], f16, kind="ExternalInput").ap()
    de_d = nc.dram_tensor("de", [P, GPP * 512], f16, kind="ExternalInput").ap()
    xm_d = nc.dram_tensor("xm", [P, GPP * 32], f16, kind="ExternalInput").ap()
    m1_d = nc.dram_tensor("m1", [P, GPP * 16], f16, kind="ExternalInput").ap()
    w_d = nc.dram_tensor("w", [P, 2 * GPP], f16, kind="ExternalInput").ap()
    o_d = nc.dram_tensor("o", [P, 2], f32, kind="ExternalOutput").ap()

    with tile.TileContext(nc) as tc:
        with (
            tc.tile_pool(name="work", bufs=1) as work,
            tc.tile_pool(name="acc", bufs=1) as acc,
        ):
            xm_all = acc.tile([P, GPP * 32], f16)
            m1_sb = acc.tile([P, GPP * 16], f16)
            w_sb = acc.tile([P, 2 * GPP], f16)
            tab1 = acc.tile([P, GPP * 16], f16)   # (Ax+Bm0) per (g, j)
            dcol1 = acc.tile([P, GPP * 32], f16)  # colsums per (g, q, j)

            in_dmas = []
            in_dmas.append(nc.sync.dma_start(out=xm_all[:, :], in_=xm_d))
            ch_t = []
            off = 0
            for ci, g in enumerate(CHUNKS):
                ta = acc.tile([P, g * 512], f16, tag=f"a{ci}")
                in_dmas.append(nc.sync.dma_start(
                    out=ta[:, :],
                    in_=ab_d[:, off * 512:(off + g) * 512]))
                td = acc.tile([P, g * 512], f16, tag=f"d{ci}")
                in_dmas.append(nc.sync.dma_start(
                    out=td[:, :],
                    in_=de_d[:, off * 512:(off + g) * 512]))
                ch_t.append((ta, td, off, g))
                off += g
            # epilogue-only data rides the (starved) ACT ring
            in_dmas.append(nc.scalar.dma_start(out=m1_sb[:, :], in_=m1_d))
            in_dmas.append(nc.scalar.dma_start(out=w_sb[:, :], in_=w_d))

            for ci, (ta, td, off, g) in enumerate(ch_t):
                ab_sb = ta[:, 0:g * 512]
                de_sb = td[:, 0:g * 512]

                # products P[g, j, m, k] = AB * xm (xm bcast over j)
                pt = work.tile([P, 14 * 512], f16, tag="pt")
                p5 = pt[:, 0:g * 512].rearrange(
                    "p (g j m k) -> p g j m k", g=g, j=16, m=2, k=16)
                ab5 = ab_sb.rearrange("p (g j m k) -> p g j m k",
                                      g=g, j=16, m=2, k=16)
                xm_bc = (xm_all[:, off * 32:(off + g) * 32]
                         .rearrange("p (g m k) -> p g m k", g=g, m=2, k=16)
                         .unsqueeze(2).broadcast_to((P, g, 16, 2, 16)))
                _lab(nc.vector.tensor_mul(out=p5, in0=ab5, in1=xm_bc),
                     f'prod{ci}')

                # m-fold then k-folds 16 -> 1
                t1 = work.tile([P, 14 * 256], f16, tag="t1")
                t1r = t1[:, 0:g * 256].rearrange(
                    "p (g j k) -> p g j k", g=g, j=16, k=16)
                _lab(nc.vector.tensor_add(out=t1r, in0=p5[:, :, :, 0],
                                          in1=p5[:, :, :, 1]), f'mfold{ci}')
                t2 = work.tile([P, 14 * 128], f16, tag="t2")
                t2r = t2[:, 0:g * 128].rearrange(
                    "p (g j k) -> p g j k", g=g, j=16, k=8)
                _lab(nc.vector.tensor_add(out=t2r, in0=t1r[:, :, :, 0:8],
                                          in1=t1r[:, :, :, 8:16]), f'kL2_{ci}')
                t4 = work.tile([P, 14 * 64], f16, tag="t4")
                t4r = t4[:, 0:g * 64].rearrange(
                    "p (g j k) -> p g j k", g=g, j=16, k=4)
                _lab(nc.vector.tensor_add(out=t4r, in0=t2r[:, :, :, 0:4],
                                          in1=t2r[:, :, :, 4:8]), f'kL3_{ci}')
                tw = work.tile([P, 14 * 32], f16, tag="tw")
                twr = tw[:, 0:g * 32].rearrange(
                    "p (g j k) -> p g j k", g=g, j=16, k=2)
                _lab(nc.vector.tensor_add(out=twr, in0=t4r[:, :, :, 0:2],
                                          in1=t4r[:, :, :, 2:4]), f'kL4_{ci}')
                tab_s = tab1[:, off * 16:(off + g) * 16].rearrange(
                    "p (g j) -> p g j", g=g, j=16)
                _lab(nc.vector.tensor_add(out=tab_s, in0=twr[:, :, :, 0],
                                          in1=twr[:, :, :, 1]), f'kL5_{ci}')

                # DE reduce (i 16 -> 1)
                de5 = de_sb.rearrange("p (g q j i) -> p g q j i",
                                      g=g, q=2, j=16, i=16)
                d8 = work.tile([P, 14 * 256], f16, tag="d8")
                d8r = d8[:, 0:g * 256].rearrange(
                    "p (g q j i) -> p g q j i", g=g, q=2, j=16, i=8)
                _lab(nc.vector.tensor_add(out=d8r, in0=de5[:, :, :, :, 0:8],
                                          in1=de5[:, :, :, :, 8:16]),
                     f'deL1_{ci}')
                d4 = work.tile([P, 14 * 128], f16, tag="d4")
                d4r = d4[:, 0:g * 128].rearrange(
                    "p (g q j i) -> p g q j i", g=g, q=2, j=16, i=4)
                _lab(nc.vector.tensor_add(out=d4r, in0=d8r[:, :, :, :, 0:4],
                                          in1=d8r[:, :, :, :, 4:8]),
                     f'd4_{ci}')
                d2 = work.tile([P, 14 * 64], f16, tag="d2")
                d2r = d2[:, 0:g * 64].rearrange(
                    "p (g q j i) -> p g q j i", g=g, q=2, j=16, i=2)
                _lab(nc.vector.tensor_add(out=d2r, in0=d4r[:, :, :, :, 0:2],
                                          in1=d4r[:, :, :, :, 2:4]),
                     f'd2_{ci}')
                dc_s = dcol1[:, off * 32:(off + g) * 32].rearrange(
                    "p (g q j) -> p g q j", g=g, q=2, j=16)
                _lab(nc.vector.tensor_add(out=dc_s, in0=d2r[:, :, :, :, 0],
                                          in1=d2r[:, :, :, :, 1]),
                     f'deL4_{ci}')

            # ---- epilogue ----
            # R[m, g, j]: m=0 -> tab1 * dcolD, m=1 -> m1 * dcolE
            dv = dcol1[:, :].rearrange("p (g q j) -> p g q j",
                                       g=GPP, q=2, j=16)
            r = acc.tile([P, 2 * GPP * 16], f16)
            rv = r[:, :].rearrange("p (m g j) -> p m g j",
                                   m=2, g=GPP, j=16)
            _lab(nc.vector.tensor_mul(
                out=rv[:, 0],
                in0=tab1[:, :].rearrange("p (g j) -> p g j", g=GPP, j=16),
                in1=dv[:, :, 0]), 'R0')
            _lab(nc.vector.tensor_mul(
                out=rv[:, 1],
                in0=m1_sb[:, :].rearrange("p (g j) -> p g j", g=GPP, j=16),
                in1=dv[:, :, 1]), 'R1')
            # fold m then j: 16 -> 8 -> 4 -> 2 -> 1
            sm = acc.tile([P, GPP * 16], f16)
            nc.vector.tensor_add(out=sm[:, :], in0=r[:, 0:GPP * 16],
                                 in1=r[:, GPP * 16:2 * GPP * 16])
            smr = sm[:, :].rearrange("p (g j) -> p g j", g=GPP, j=16)
            s8 = acc.tile([P, GPP * 8], f16)
            s8r = s8[:, :].rearrange("p (g j) -> p g j", g=GPP, j=8)
            nc.vector.tensor_add(out=s8r, in0=smr[:, :, 0:8],
                                 in1=smr[:, :, 8:16])
            s4 = acc.tile([P, GPP * 4], f16)
            s4r = s4[:, :].rearrange("p (g j) -> p g j", g=GPP, j=4)
            nc.vector.tensor_add(out=s4r, in0=s8r[:, :, 0:4],
                                 in1=s8r[:, :, 4:8])
            s2 = acc.tile([P, GPP * 2], f16)
            s2r = s2[:, :].rearrange("p (g j) -> p g j", g=GPP, j=2)
            nc.vector.tensor_add(out=s2r, in0=s4r[:, :, 0:2],
                                 in1=s4r[:, :, 2:4])
            s1 = acc.tile([P, GPP], f16)
            nc.vector.tensor_add(
                out=s1[:, :].rearrange("p g -> p g"),
                in0=s2r[:, :, 0], in1=s2r[:, :, 1])

            # head: o[:, c] = sum_g s1[:, g] * w[:, c, g]  (f32 accumulate)
            hp = acc.tile([P, 2 * GPP], f16)
            hpv = hp[:, :].rearrange("p (c g) -> p c g", c=2, g=GPP)
            nc.vector.tensor_mul(
                out=hpv,
                in0=w_sb[:, :].rearrange("p (c g) -> p c g", c=2, g=GPP),
                in1=s1[:, :].rearrange("p g -> p g").unsqueeze(1)
                .broadcast_to((P, 2, GPP)))
            o_sb = acc.tile([P, 2], f32)
            nc.vector.tensor_reduce(
                out=o_sb[:, :].rearrange("p c -> p c"),
                in_=hpv, axis=mybir.AxisListType.X, op=ADD)
            nc.sync.dma_start(out=o_d, in_=o_sb[:, :])

    nc._input_dma_names = {i.ins.name for i in in_dmas}
    return nc


def _get_nc():
    if "nc" not in _cache:
        _cache["nc"] = _build_nc()
    return _cache["nc"]


def _shard(x):
    """(N, ...) f32 -> (CORES, 128, GPP, ...) fp16, zero padded.
    Node mapping: n = (core*128 + p)*GPP + g."""
    out = np.zeros((NP,) + x.shape[1:], np.float16)
    out[:N] = x.astype(np.float16)
    return out.reshape((CORES, P, GPP) + x.shape[1:])


def kernel(h0, cw0, mw0, cw1, mw1,
           msg0_r0, msg0_r1, msg0_r2,
           msg1_r0, msg1_r1, msg1_r2,
           w_pred, b_pred):
    from concourse.bass_utils import run_bass_kernel_spmd

    nc = _get_nc()
    if not _cache.get("split_done"):
        import concourse.mybir as mybir
        _unleash_input_dmas(nc, nc._input_dma_names)
        _split_multiwait(nc, mybir)
        _cache["split_done"] = True

    A = np.asarray(cw0[0], np.float32)
    B = np.asarray(mw0[0], np.float32)
    D = np.asarray(cw1[0], np.float32)
    E = np.asarray(mw1[0], np.float32)

    # ab[n, j, m, k] = {A,B}[n, j, k]; de[n, q, j, i] = {D,E}[n, i, j]
    AB = _shard(np.stack([A, B], axis=2)).reshape(CORES, P, GPP * 512)
    DE = _shard(np.stack([D.transpose(0, 2, 1), E.transpose(0, 2, 1)],
                         axis=1)).reshape(CORES, P, GPP * 512)

    XM = _shard(np.stack([np.asarray(h0, np.float32)[..., 0],
                          np.asarray(msg0_r0, np.float32)[..., 0]],
                         axis=1)).reshape(CORES, P, GPP * 32)
    M1 = _shard(np.asarray(msg1_r0, np.float32)[..., 0]
                ).reshape(CORES, P, GPP * 16)

    wp = np.zeros((2, NP), np.float32)
    wp[:, :N] = np.asarray(w_pred, np.float32)
    W = np.ascontiguousarray(
        wp.reshape(2, CORES, P, GPP).transpose(1, 2, 0, 3)
        .reshape(CORES, P, 2 * GPP)).astype(np.float16)

    in_maps = [
        {"ab": np.ascontiguousarray(AB[i]),
         "de": np.ascontiguousarray(DE[i]),
         "xm": np.ascontiguousarray(XM[i]),
         "m1": np.ascontiguousarray(M1[i]),
         "w": np.ascontiguousarray(W[i])}
        for i in range(CORES)
    ]
    res = run_bass_kernel_spmd(nc, in_maps, list(range(CORES)), trace=TRACE)
    _cache["last_res"] = res
    partial = np.zeros(2, np.float64)
    for i in range(CORES):
        partial += res.results[i]["o"].astype(np.float64).sum(axis=0)
    out = (partial + np.asarray(b_pred, np.float64)).astype(np.float32)
    return out.reshape(1, 2)


# revision 36
# speedup vs baseline: 1.2699x; 1.0050x over previous
"""CartesianMACE rank-0 fused kernel for 8 trn2 NeuronCores (fp16, all-DVE).

The reference's ranks 1 and 2 never reach the output (each rank is mixed
independently and the head reads only h[0]), so only the rank-0 slices of
cw0/mw0/cw1/mw1 plus h0/msg0_r0/msg1_r0/w_pred/b_pred are needed.

Per node n (16x16 mats A=cw0[0,n], B=mw0[0,n], D=cw1[0,n], E=mw1[0,n];
16-vecs x=h0[n], m0=msg0_r0[n], m1=msg1_r0[n]):
    s[n] = colsum(D) . (A x + B m0) + colsum(E) . m1
    out  = [sum_n s[n] w_pred[0,n], sum_n s[n] w_pred[1,n]] + b_pred

Design notes (HW-measured on this device):
- fp16 halves HBM bytes and doubles DVE tensor_tensor throughput (2x_1P
  engages for contiguous, strided-even-run, and broadcast operands).
- All reductions are binary fold trees of tensor_add at 2x; tensor_reduce
  and pool only have 1x uops (pool doesn't even encode on this walrus).
- GPSIMD shares its SBUF port with the DVE ("POOL slot"): concurrent
  gpsimd work slows DVE ops up to 4-6x, so gpsimd is net-negative for this
  DVE-bound kernel — everything runs on the DVE.
- The two HWDGE rings share SDMA engines unfairly (SP starves ACT), so all
  latency-critical loads go on the SP ring in consumption order; only the
  epilogue-only m1/w go on ACT. ab/de are interleaved chunk-wise in one
  dram tensor so each chunk arrives as one large DMA.
- Tile's scheduler ties input-DMA issue to compute progress; those waits
  are stripped post-hoc (_unleash_input_dmas) — destinations are
  write-once tiles and readers wait on the DMA-completion sems.

Sharding: data-parallel over nodes. 50000 nodes padded to 50176 =
8 cores x 128 partitions x 49 groups. Per-core [128, 2] f32 partial head
outputs are summed on host (the all-reduce of the head).
"""

import sys

for _p in ("/opt/trn_rl_repo", "/root/.axon_site/_ro/trn_rl_repo"):
    if _p not in sys.path:
        sys.path.append(_p)

import numpy as np

N, CH = 50000, 16
CORES = 8
P = 128
GPP = 49                  # node groups per partition
NP = CORES * P * GPP      # 50176 padded nodes
CHUNKS = (4, 10, 14, 14, 7)   # node groups per compute chunk / DMA slice

_cache = {}
TRACE = False


def _split_multiwait(nc, mybir):
    """This walrus build accepts a single sync-wait per instruction, but Tile
    attaches one wait per producer proc. Split: keep the last wait on the
    instruction and hoist the rest onto fresh same-engine Drain carriers
    inserted immediately before it (engines execute their stream in-order,
    so semantics are identical)."""
    for fn in nc.m.functions:
        for bb in fn.blocks:
            insts = bb.instructions  # live list
            i = 0
            while i < len(insts):
                ins = insts[i]
                si = ins.sync_info
                if si is not None and len(si.on_wait) > 1:
                    waits = list(si.on_wait)
                    ins.sync_info = mybir.SyncInfo(
                        on_wait=waits[-1:], on_update=list(si.on_update))
                    for k, w in enumerate(waits[:-1]):
                        insts.insert(i + k, mybir.InstDrain(
                            name=f"{ins.name}_w{k}", opcode="Drain",
                            engine=ins.engine, ins=[], outs=[],
                            sync_info=mybir.SyncInfo(on_wait=[w], on_update=[]),
                        ))
                    i += len(waits) - 1
                i += 1


def _unleash_input_dmas(nc, names):
    """Strip scheduling-artifact waits from the input loads: keep only
    same-lane sem-reuse waits (a wait on the sem the DMA itself updates,
    which orders same-lane completions); engine-progress and cross-lane
    chain waits only serialize prefetch. Destinations are write-once tiles
    and every reader waits on the DMA-completion sems, so this is safe."""
    for fn in nc.m.functions:
        for bb in fn.blocks:
            for ins in bb.instructions:
                if ins.name in names and ins.sync_info is not None:
                    si = ins.sync_info
                    upd = {u.ant_name for u in si.on_update}
                    keep = [w for w in si.on_wait if w.ant_name in upd]
                    ins.sync_info = type(si)(
                        on_wait=keep, on_update=list(si.on_update))


def _build_nc():
    import concourse.bass as bass
    import concourse.tile as tile
    import concourse.mybir as mybir

    f16 = mybir.dt.float16
    f32 = mybir.dt.float32
    ADD = mybir.AluOpType.add

    nc = bass.Bass("TRN2", target_bir_lowering=False, debug=False,
                   num_devices=CORES)
    nc._opmap = {}

    def _lab(r, name):
        try:
            nc._opmap[r.ins.name] = name
        except Exception:
            pass
        return r

    # per-partition free layouts:
    #   ab: g, j(16), m(2), k(16)   de: g, q(2), j(16), i(16)
    #   xm: g, m(2), k(16)          m1: g, j(16)        w: c(2), g(GPP)
    ab_d = nc.dram_tensor("ab", [P, GPP * 512], f16, kind="ExternalInput").ap()
    de_d = nc.dram_tensor("de", [P, GPP * 512], f16, kind="ExternalInput").ap()
    xm_d = nc.dram_tensor("xm", [P, GPP * 32], f16, kind="ExternalInput").ap()
    m1_d = nc.dram_tensor("m1", [P, GPP * 16], f16, kind="ExternalInput").ap()
    w_d = nc.dram_tensor("w", [P, 2 * GPP], f16, kind="ExternalInput").ap()
    o_d = nc.dram_tensor("o", [P, 2], f32, kind="ExternalOutput").ap()

    with tile.TileContext(nc) as tc:
        with (
            tc.tile_pool(name="work", bufs=1) as work,
            tc.tile_pool(name="acc", bufs=1) as acc,
        ):
            xm_all = acc.tile([P, GPP * 32], f16)
            m1_sb = acc.tile([P, GPP * 16], f16)
            w_sb = acc.tile([P, 2 * GPP], f16)
            tab1 = acc.tile([P, GPP * 16], f16)   # (Ax+Bm0) per (g, j)
            dcol1 = acc.tile([P, GPP * 32], f16)  # colsums per (g, q, j)

            in_dmas = []
            in_dmas.append(nc.sync.dma_start(out=xm_all[:, :], in_=xm_d))
            # ab loads run one chunk ahead of de loads: products(c) (first
            # DVE op of chunk c) then never waits mid-stream, and de(c)
            # lands while chunk c's AB-side folds execute.
            ch_t = []
            off = 0
            for ci, g in enumerate(CHUNKS):
                ta = acc.tile([P, g * 512], f16, tag=f"a{ci}")
                td = acc.tile([P, g * 512], f16, tag=f"d{ci}")
                ch_t.append((ta, td, off, g))
                off += g
            order = []
            for ci in range(len(CHUNKS)):
                order.append((ci, True))          # ab(ci)
                if ci >= 1:
                    order.append((ci - 1, False))  # de(ci-1)
            order.append((len(CHUNKS) - 1, False))
            for ci, is_ab in order:
                ta, td, off, g = ch_t[ci]
                if is_ab:
                    in_dmas.append(nc.sync.dma_start(
                        out=ta[:, :],
                        in_=ab_d[:, off * 512:(off + g) * 512]))
                else:
                    in_dmas.append(nc.sync.dma_start(
                        out=td[:, :],
                        in_=de_d[:, off * 512:(off + g) * 512]))
            # epilogue-only data rides the (starved) ACT ring
            in_dmas.append(nc.scalar.dma_start(out=m1_sb[:, :], in_=m1_d))
            in_dmas.append(nc.scalar.dma_start(out=w_sb[:, :], in_=w_d))

            for ci, (ta, td, off, g) in enumerate(ch_t):
                ab_sb = ta[:, 0:g * 512]
                de_sb = td[:, 0:g * 512]

                # products P[g, j, m, k] = AB * xm (xm bcast over j)
                pt = work.tile([P, 14 * 512], f16, tag="pt")
                p5 = pt[:, 0:g * 512].rearrange(
                    "p (g j m k) -> p g j m k", g=g, j=16, m=2, k=16)
                ab5 = ab_sb.rearrange("p (g j m k) -> p g j m k",
                                      g=g, j=16, m=2, k=16)
                xm_bc = (xm_all[:, off * 32:(off + g) * 32]
                         .rearrange("p (g m k) -> p g m k", g=g, m=2, k=16)
                         .unsqueeze(2).broadcast_to((P, g, 16, 2, 16)))
                _lab(nc.vector.tensor_mul(out=p5, in0=ab5, in1=xm_bc),
                     f'prod{ci}')

                # m-fold then k-folds 16 -> 1
                t1 = work.tile([P, 14 * 256], f16, tag="t1")
                t1r = t1[:, 0:g * 256].rearrange(
                    "p (g j k) -> p g j k", g=g, j=16, k=16)
                _lab(nc.vector.tensor_add(out=t1r, in0=p5[:, :, :, 0],
                                          in1=p5[:, :, :, 1]), f'mfold{ci}')
                t2 = work.tile([P, 14 * 128], f16, tag="t2")
                t2r = t2[:, 0:g * 128].rearrange(
                    "p (g j k) -> p g j k", g=g, j=16, k=8)
                _lab(nc.vector.tensor_add(out=t2r, in0=t1r[:, :, :, 0:8],
                                          in1=t1r[:, :, :, 8:16]), f'kL2_{ci}')
                t4 = work.tile([P, 14 * 64], f16, tag="t4")
                t4r = t4[:, 0:g * 64].rearrange(
                    "p (g j k) -> p g j k", g=g, j=16, k=4)
                _lab(nc.vector.tensor_add(out=t4r, in0=t2r[:, :, :, 0:4],
                                          in1=t2r[:, :, :, 4:8]), f'kL3_{ci}')
                tw = work.tile([P, 14 * 32], f16, tag="tw")
                twr = tw[:, 0:g * 32].rearrange(
                    "p (g j k) -> p g j k", g=g, j=16, k=2)
                _lab(nc.vector.tensor_add(out=twr, in0=t4r[:, :, :, 0:2],
                                          in1=t4r[:, :, :, 2:4]), f'kL4_{ci}')
                tab_s = tab1[:, off * 16:(off + g) * 16].rearrange(
                    "p (g j) -> p g j", g=g, j=16)
                _lab(nc.vector.tensor_add(out=tab_s, in0=twr[:, :, :, 0],
                                          in1=twr[:, :, :, 1]), f'kL5_{ci}')

                # DE reduce (i 16 -> 1)
                de5 = de_sb.rearrange("p (g q j i) -> p g q j i",
                                      g=g, q=2, j=16, i=16)
                d8 = work.tile([P, 14 * 256], f16, tag="d8")
                d8r = d8[:, 0:g * 256].rearrange(
                    "p (g q j i) -> p g q j i", g=g, q=2, j=16, i=8)
                _lab(nc.vector.tensor_add(out=d8r, in0=de5[:, :, :, :, 0:8],
                                          in1=de5[:, :, :, :, 8:16]),
                     f'deL1_{ci}')
                d4 = work.tile([P, 14 * 128], f16, tag="d4")
                d4r = d4[:, 0:g * 128].rearrange(
                    "p (g q j i) -> p g q j i", g=g, q=2, j=16, i=4)
                _lab(nc.vector.tensor_add(out=d4r, in0=d8r[:, :, :, :, 0:4],
                                          in1=d8r[:, :, :, :, 4:8]),
                     f'd4_{ci}')
                d2 = work.tile([P, 14 * 64], f16, tag="d2")
                d2r = d2[:, 0:g * 64].rearrange(
                    "p (g q j i) -> p g q j i", g=g, q=2, j=16, i=2)
                _lab(nc.vector.tensor_add(out=d2r, in0=d4r[:, :, :, :, 0:2],
                                          in1=d4r[:, :, :, :, 2:4]),
                     f'd2_{ci}')
                dc_s = dcol1[:, off * 32:(off + g) * 32].rearrange(
                    "p (g q j) -> p g q j", g=g, q=2, j=16)
                _lab(nc.vector.tensor_add(out=dc_s, in0=d2r[:, :, :, :, 0],
                                          in1=d2r[:, :, :, :, 1]),
                     f'deL4_{ci}')

            # ---- epilogue ----
            # R[m, g, j]: m=0 -> tab1 * dcolD, m=1 -> m1 * dcolE
            dv = dcol1[:, :].rearrange("p (g q j) -> p g q j",
                                       g=GPP, q=2, j=16)
            r = acc.tile([P, 2 * GPP * 16], f16)
            rv = r[:, :].rearrange("p (m g j) -> p m g j",
                                   m=2, g=GPP, j=16)
            _lab(nc.vector.tensor_mul(
                out=rv[:, 0],
                in0=tab1[:, :].rearrange("p (g j) -> p g j", g=GPP, j=16),
                in1=dv[:, :, 0]), 'R0')
            _lab(nc.vector.tensor_mul(
                out=rv[:, 1],
                in0=m1_sb[:, :].rearrange("p (g j) -> p g j", g=GPP, j=16),
                in1=dv[:, :, 1]), 'R1')
            # fold m then j: 16 -> 8 -> 4 -> 2 -> 1
            sm = acc.tile([P, GPP * 16], f16)
            nc.vector.tensor_add(out=sm[:, :], in0=r[:, 0:GPP * 16],
                                 in1=r[:, GPP * 16:2 * GPP * 16])
            smr = sm[:, :].rearrange("p (g j) -> p g j", g=GPP, j=16)
            s8 = acc.tile([P, GPP * 8], f16)
            s8r = s8[:, :].rearrange("p (g j) -> p g j", g=GPP, j=8)
            nc.vector.tensor_add(out=s8r, in0=smr[:, :, 0:8],
                                 in1=smr[:, :, 8:16])
            s4 = acc.tile([P, GPP * 4], f16)
            s4r = s4[:, :].rearrange("p (g j) -> p g j", g=GPP, j=4)
            nc.vector.tensor_add(out=s4r, in0=s8r[:, :, 0:4],
                                 in1=s8r[:, :, 4:8])
            s2 = acc.tile([P, GPP * 2], f16)
            s2r = s2[:, :].rearrange("p (g j) -> p g j", g=GPP, j=2)
            nc.vector.tensor_add(out=s2r, in0=s4r[:, :, 0:2],
                                 in1=s4r[:, :, 2:4])
            s1 = acc.tile([P, GPP], f16)
            nc.vector.tensor_add(
                out=s1[:, :].rearrange("p g -> p g"),
                in0=s2r[:, :, 0], in1=s2r[:, :, 1])

            # head: o[:, c] = sum_g s1[:, g] * w[:, c, g]  (f32 accumulate)
            hp = acc.tile([P, 2 * GPP], f16)
            hpv = hp[:, :].rearrange("p (c g) -> p c g", c=2, g=GPP)
            nc.vector.tensor_mul(
                out=hpv,
                in0=w_sb[:, :].rearrange("p (c g) -> p c g", c=2, g=GPP),
                in1=s1[:, :].rearrange("p g -> p g").unsqueeze(1)
                .broadcast_to((P, 2, GPP)))
            o_sb = acc.tile([P, 2], f32)
            nc.vector.tensor_reduce(
                out=o_sb[:, :].rearrange("p c -> p c"),
                in_=hpv, axis=mybir.AxisListType.X, op=ADD)
            nc.sync.dma_start(out=o_d, in_=o_sb[:, :])

    nc._input_dma_names = {i.ins.name for i in in_dmas}
    return nc


def _get_nc():
    if "nc" not in _cache:
        _cache["nc"] = _build_nc()
    return _cache["nc"]


def _shard(x):
    """(N, ...) f32 -> (CORES, 128, GPP, ...) fp16, zero padded.
    Node mapping: n = (core*128 + p)*GPP + g."""
    out = np.zeros((NP,) + x.shape[1:], np.float16)
    out[:N] = x.astype(np.float16)
    return out.reshape((CORES, P, GPP) + x.shape[1:])


def kernel(h0, cw0, mw0, cw1, mw1,
           msg0_r0, msg0_r1, msg0_r2,
           msg1_r0, msg1_r1, msg1_r2,
           w_pred, b_pred):
    from concourse.bass_utils import run_bass_kernel_spmd

    nc = _get_nc()
    if not _cache.get("split_done"):
        import concourse.mybir as mybir
        _unleash_input_dmas(nc, nc._input_dma_names)
        _split_multiwait(nc, mybir)
        _cache["split_done"] = True

    A = np.asarray(cw0[0], np.float32)
    B = np.asarray(mw0[0], np.float32)
    D = np.asarray(cw1[0], np.float32)
    E = np.asarray(mw1[0], np.float32)

    # ab[n, j, m, k] = {A,B}[n, j, k]; de[n, q, j, i] = {D,E}[n, i, j]
    AB = _shard(np.stack([A, B], axis=2)).reshape(CORES, P, GPP * 512)
    DE = _shard(np.stack([D.transpose(0, 2, 1), E.transpose(0, 2, 1)],
                         axis=1)).reshape(CORES, P, GPP * 512)

    XM = _shard(np.stack([np.asarray(h0, np.float32)[..., 0],
                          np.asarray(msg0_r0, np.float32)[..., 0]],
                         axis=1)).reshape(CORES, P, GPP * 32)
    M1 = _shard(np.asarray(msg1_r0, np.float32)[..., 0]
                ).reshape(CORES, P, GPP * 16)

    wp = np.zeros((2, NP), np.float32)
    wp[:, :N] = np.asarray(w_pred, np.float32)
    W = np.ascontiguousarray(
        wp.reshape(2, CORES, P, GPP).transpose(1, 2, 0, 3)
        .reshape(CORES, P, 2 * GPP)).astype(np.float16)

    in_maps = [
        {"ab": np.ascontiguousarray(AB[i]),
         "de": np.ascontiguousarray(DE[i]),
         "xm": np.ascontiguousarray(XM[i]),
         "m1": np.ascontiguousarray(M1[i]),
         "w": np.ascontiguousarray(W[i])}
        for i in range(CORES)
    ]
    res = run_bass_kernel_spmd(nc, in_maps, list(range(CORES)), trace=TRACE)
    _cache["last_res"] = res
    partial = np.zeros(2, np.float64)
    for i in range(CORES):
        partial += res.results[i]["o"].astype(np.float64).sum(axis=0)
    out = (partial + np.asarray(b_pred, np.float64)).astype(np.float32)
    return out.reshape(1, 2)


# revision 39
# speedup vs baseline: 1.2893x; 1.0152x over previous
"""CartesianMACE rank-0 fused kernel for 8 trn2 NeuronCores (fp16, all-DVE).

The reference's ranks 1 and 2 never reach the output (each rank is mixed
independently and the head reads only h[0]), so only the rank-0 slices of
cw0/mw0/cw1/mw1 plus h0/msg0_r0/msg1_r0/w_pred/b_pred are needed.

Per node n (16x16 mats A=cw0[0,n], B=mw0[0,n], D=cw1[0,n], E=mw1[0,n];
16-vecs x=h0[n], m0=msg0_r0[n], m1=msg1_r0[n]):
    s[n] = colsum(D) . (A x + B m0) + colsum(E) . m1
    out  = [sum_n s[n] w_pred[0,n], sum_n s[n] w_pred[1,n]] + b_pred

Design notes (HW-measured on this device):
- fp16 halves HBM bytes and doubles DVE tensor_tensor throughput (2x_1P
  engages for contiguous, strided-even-run, and broadcast operands).
- All reductions are binary fold trees of tensor_add at 2x; tensor_reduce
  and pool only have 1x uops (pool doesn't even encode on this walrus).
- GPSIMD shares its SBUF port with the DVE ("POOL slot"): concurrent
  gpsimd work slows DVE ops up to 4-6x, so gpsimd is net-negative for this
  DVE-bound kernel — everything runs on the DVE.
- The two HWDGE rings share SDMA engines unfairly (SP starves ACT), so all
  latency-critical loads go on the SP ring in consumption order; only the
  epilogue-only m1/w go on ACT. ab/de are interleaved chunk-wise in one
  dram tensor so each chunk arrives as one large DMA.
- Tile's scheduler ties input-DMA issue to compute progress; those waits
  are stripped post-hoc (_unleash_input_dmas) — destinations are
  write-once tiles and readers wait on the DMA-completion sems.

Sharding: data-parallel over nodes. 50000 nodes padded to 50176 =
8 cores x 128 partitions x 49 groups. Per-core [128, 2] f32 partial head
outputs are summed on host (the all-reduce of the head).
"""

import sys

for _p in ("/opt/trn_rl_repo", "/root/.axon_site/_ro/trn_rl_repo"):
    if _p not in sys.path:
        sys.path.append(_p)

import numpy as np

N, CH = 50000, 16
CORES = 8
P = 128
GPP = 49                  # node groups per partition
NP = CORES * P * GPP      # 50176 padded nodes
CHUNKS = (4, 10, 14, 14, 7)   # node groups per compute chunk / DMA slice

_cache = {}
TRACE = False


def _split_multiwait(nc, mybir):
    """This walrus build accepts a single sync-wait per instruction, but Tile
    attaches one wait per producer proc. Split: keep the last wait on the
    instruction and hoist the rest onto fresh same-engine Drain carriers
    inserted immediately before it (engines execute their stream in-order,
    so semantics are identical)."""
    for fn in nc.m.functions:
        for bb in fn.blocks:
            insts = bb.instructions  # live list
            i = 0
            while i < len(insts):
                ins = insts[i]
                si = ins.sync_info
                if si is not None and len(si.on_wait) > 1:
                    waits = list(si.on_wait)
                    ins.sync_info = mybir.SyncInfo(
                        on_wait=waits[-1:], on_update=list(si.on_update))
                    for k, w in enumerate(waits[:-1]):
                        insts.insert(i + k, mybir.InstDrain(
                            name=f"{ins.name}_w{k}", opcode="Drain",
                            engine=ins.engine, ins=[], outs=[],
                            sync_info=mybir.SyncInfo(on_wait=[w], on_update=[]),
                        ))
                    i += len(waits) - 1
                i += 1


def _unleash_input_dmas(nc, names):
    """Strip scheduling-artifact waits from the input loads: keep only
    same-lane sem-reuse waits (a wait on the sem the DMA itself updates,
    which orders same-lane completions); engine-progress and cross-lane
    chain waits only serialize prefetch. Destinations are write-once tiles
    and every reader waits on the DMA-completion sems, so this is safe."""
    for fn in nc.m.functions:
        for bb in fn.blocks:
            for ins in bb.instructions:
                if ins.name in names and ins.sync_info is not None:
                    si = ins.sync_info
                    upd = {u.ant_name for u in si.on_update}
                    keep = [w for w in si.on_wait if w.ant_name in upd]
                    ins.sync_info = type(si)(
                        on_wait=keep, on_update=list(si.on_update))


def _build_nc():
    import concourse.bass as bass
    import concourse.tile as tile
    import concourse.mybir as mybir

    f16 = mybir.dt.float16
    f32 = mybir.dt.float32
    ADD = mybir.AluOpType.add

    nc = bass.Bass("TRN2", target_bir_lowering=False, debug=False,
                   num_devices=CORES)
    nc._opmap = {}

    def _lab(r, name):
        try:
            nc._opmap[r.ins.name] = name
        except Exception:
            pass
        return r

    # per-partition free layouts:
    #   ab: g, j(16), m(2), k(16)   de: g, q(2), j(16), i(16)
    #   xm: g, m(2), k(16)          m1: g, j(16)        w: c(2), g(GPP)
    ab_d = nc.dram_tensor("ab", [P, GPP * 512], f16, kind="ExternalInput").ap()
    de_d = nc.dram_tensor("de", [P, GPP * 512], f16, kind="ExternalInput").ap()
    xm_d = nc.dram_tensor("xm", [P, GPP * 32], f16, kind="ExternalInput").ap()
    m1_d = nc.dram_tensor("m1", [P, GPP * 16], f16, kind="ExternalInput").ap()
    w_d = nc.dram_tensor("w", [P, 2 * GPP], f16, kind="ExternalInput").ap()
    o_d = nc.dram_tensor("o", [P, 2], f32, kind="ExternalOutput").ap()

    with tile.TileContext(nc) as tc:
        with (
            tc.tile_pool(name="work", bufs=1) as work,
            tc.tile_pool(name="acc", bufs=1) as acc,
        ):
            xm_all = acc.tile([P, GPP * 32], f16)
            m1_sb = acc.tile([P, GPP * 16], f16)
            w_sb = acc.tile([P, 2 * GPP], f16)
            tab1 = acc.tile([P, GPP * 16], f16)   # (Ax+Bm0) per (g, j)
            dcol1 = acc.tile([P, GPP * 32], f16)  # colsums per (g, q, j)

            in_dmas = []
            # ab loads run one chunk ahead of de loads: products(c) (first
            # DVE op of chunk c) then never waits mid-stream, and de(c)
            # lands while chunk c's AB-side folds execute.
            ch_t = []
            off = 0
            for ci, g in enumerate(CHUNKS):
                ta = acc.tile([P, g * 512], f16, tag=f"a{ci}")
                td = acc.tile([P, g * 512], f16, tag=f"d{ci}")
                ch_t.append((ta, td, off, g))
                off += g
            order = []
            for ci in range(len(CHUNKS)):
                order.append((ci, True))          # ab(ci)
                if ci >= 1:
                    order.append((ci - 1, False))  # de(ci-1)
            order.append((len(CHUNKS) - 1, False))
            # chunk 0's de load precedes xm: the DVE then opens with chunk
            # 0's DE chain (needs only ab0+de0) while xm is still landing
            order.insert(1, (0, False))
            del order[order.index((0, False), 2)]
            for ci, is_ab in order:
                ta, td, off, g = ch_t[ci]
                if is_ab:
                    in_dmas.append(nc.sync.dma_start(
                        out=ta[:, :],
                        in_=ab_d[:, off * 512:(off + g) * 512]))
                else:
                    in_dmas.append(nc.sync.dma_start(
                        out=td[:, :],
                        in_=de_d[:, off * 512:(off + g) * 512]))
                    if ci == 0:
                        in_dmas.append(nc.sync.dma_start(
                            out=xm_all[:, :], in_=xm_d))
            # epilogue-only data rides the (starved) ACT ring
            in_dmas.append(nc.scalar.dma_start(out=m1_sb[:, :], in_=m1_d))
            in_dmas.append(nc.scalar.dma_start(out=w_sb[:, :], in_=w_d))

            def _de_part(ci, td, off, g):
                de_sb = td[:, 0:g * 512]
                de5 = de_sb.rearrange("p (g q j i) -> p g q j i",
                                      g=g, q=2, j=16, i=16)
                d8 = work.tile([P, 14 * 256], f16, tag="d8")
                d8r = d8[:, 0:g * 256].rearrange(
                    "p (g q j i) -> p g q j i", g=g, q=2, j=16, i=8)
                _lab(nc.vector.tensor_add(out=d8r, in0=de5[:, :, :, :, 0:8],
                                          in1=de5[:, :, :, :, 8:16]),
                     f'deL1_{ci}')
                d4 = work.tile([P, 14 * 128], f16, tag="d4")
                d4r = d4[:, 0:g * 128].rearrange(
                    "p (g q j i) -> p g q j i", g=g, q=2, j=16, i=4)
                _lab(nc.vector.tensor_add(out=d4r, in0=d8r[:, :, :, :, 0:4],
                                          in1=d8r[:, :, :, :, 4:8]),
                     f'd4_{ci}')
                d2 = work.tile([P, 14 * 64], f16, tag="d2")
                d2r = d2[:, 0:g * 64].rearrange(
                    "p (g q j i) -> p g q j i", g=g, q=2, j=16, i=2)
                _lab(nc.vector.tensor_add(out=d2r, in0=d4r[:, :, :, :, 0:2],
                                          in1=d4r[:, :, :, :, 2:4]),
                     f'd2_{ci}')
                dc_s = dcol1[:, off * 32:(off + g) * 32].rearrange(
                    "p (g q j) -> p g q j", g=g, q=2, j=16)
                _lab(nc.vector.tensor_add(out=dc_s, in0=d2r[:, :, :, :, 0],
                                          in1=d2r[:, :, :, :, 1]),
                     f'deL4_{ci}')

            for ci, (ta, td, off, g) in enumerate(ch_t):
                ab_sb = ta[:, 0:g * 512]
                if ci == 0:
                    # chunk 0 opens with the DE chain: it needs only
                    # ab0+de0 which precede xm on the DMA ring
                    _de_part(ci, td, off, g)

                # products P[g, j, m, k] = AB * xm (xm bcast over j)
                pt = work.tile([P, 14 * 512], f16, tag="pt")
                p5 = pt[:, 0:g * 512].rearrange(
                    "p (g j m k) -> p g j m k", g=g, j=16, m=2, k=16)
                ab5 = ab_sb.rearrange("p (g j m k) -> p g j m k",
                                      g=g, j=16, m=2, k=16)
                xm_bc = (xm_all[:, off * 32:(off + g) * 32]
                         .rearrange("p (g m k) -> p g m k", g=g, m=2, k=16)
                         .unsqueeze(2).broadcast_to((P, g, 16, 2, 16)))
                _lab(nc.vector.tensor_mul(out=p5, in0=ab5, in1=xm_bc),
                     f'prod{ci}')

                # m-fold then k-folds 16 -> 1
                t1 = work.tile([P, 14 * 256], f16, tag="t1")
                t1r = t1[:, 0:g * 256].rearrange(
                    "p (g j k) -> p g j k", g=g, j=16, k=16)
                _lab(nc.vector.tensor_add(out=t1r, in0=p5[:, :, :, 0],
                                          in1=p5[:, :, :, 1]), f'mfold{ci}')
                t2 = work.tile([P, 14 * 128], f16, tag="t2")
                t2r = t2[:, 0:g * 128].rearrange(
                    "p (g j k) -> p g j k", g=g, j=16, k=8)
                _lab(nc.vector.tensor_add(out=t2r, in0=t1r[:, :, :, 0:8],
                                          in1=t1r[:, :, :, 8:16]), f'kL2_{ci}')
                t4 = work.tile([P, 14 * 64], f16, tag="t4")
                t4r = t4[:, 0:g * 64].rearrange(
                    "p (g j k) -> p g j k", g=g, j=16, k=4)
                _lab(nc.vector.tensor_add(out=t4r, in0=t2r[:, :, :, 0:4],
                                          in1=t2r[:, :, :, 4:8]), f'kL3_{ci}')
                tw = work.tile([P, 14 * 32], f16, tag="tw")
                twr = tw[:, 0:g * 32].rearrange(
                    "p (g j k) -> p g j k", g=g, j=16, k=2)
                _lab(nc.vector.tensor_add(out=twr, in0=t4r[:, :, :, 0:2],
                                          in1=t4r[:, :, :, 2:4]), f'kL4_{ci}')
                tab_s = tab1[:, off * 16:(off + g) * 16].rearrange(
                    "p (g j) -> p g j", g=g, j=16)
                _lab(nc.vector.tensor_add(out=tab_s, in0=twr[:, :, :, 0],
                                          in1=twr[:, :, :, 1]), f'kL5_{ci}')

                if ci > 0:
                    _de_part(ci, td, off, g)

            # ---- epilogue ----
            # R[m, g, j]: m=0 -> tab1 * dcolD, m=1 -> m1 * dcolE
            dv = dcol1[:, :].rearrange("p (g q j) -> p g q j",
                                       g=GPP, q=2, j=16)
            r = acc.tile([P, 2 * GPP * 16], f16)
            rv = r[:, :].rearrange("p (m g j) -> p m g j",
                                   m=2, g=GPP, j=16)
            _lab(nc.vector.tensor_mul(
                out=rv[:, 0],
                in0=tab1[:, :].rearrange("p (g j) -> p g j", g=GPP, j=16),
                in1=dv[:, :, 0]), 'R0')
            _lab(nc.vector.tensor_mul(
                out=rv[:, 1],
                in0=m1_sb[:, :].rearrange("p (g j) -> p g j", g=GPP, j=16),
                in1=dv[:, :, 1]), 'R1')
            # fold m then j: 16 -> 8 -> 4 -> 2 -> 1
            sm = acc.tile([P, GPP * 16], f16)
            nc.vector.tensor_add(out=sm[:, :], in0=r[:, 0:GPP * 16],
                                 in1=r[:, GPP * 16:2 * GPP * 16])
            smr = sm[:, :].rearrange("p (g j) -> p g j", g=GPP, j=16)
            s8 = acc.tile([P, GPP * 8], f16)
            s8r = s8[:, :].rearrange("p (g j) -> p g j", g=GPP, j=8)
            nc.vector.tensor_add(out=s8r, in0=smr[:, :, 0:8],
                                 in1=smr[:, :, 8:16])
            s4 = acc.tile([P, GPP * 4], f16)
            s4r = s4[:, :].rearrange("p (g j) -> p g j", g=GPP, j=4)
            nc.vector.tensor_add(out=s4r, in0=s8r[:, :, 0:4],
                                 in1=s8r[:, :, 4:8])
            s2 = acc.tile([P, GPP * 2], f16)
            s2r = s2[:, :].rearrange("p (g j) -> p g j", g=GPP, j=2)
            nc.vector.tensor_add(out=s2r, in0=s4r[:, :, 0:2],
                                 in1=s4r[:, :, 2:4])
            s1 = acc.tile([P, GPP], f16)
            nc.vector.tensor_add(
                out=s1[:, :].rearrange("p g -> p g"),
                in0=s2r[:, :, 0], in1=s2r[:, :, 1])

            # head: o[:, c] = sum_g s1[:, g] * w[:, c, g]  (f32 accumulate)
            hp = acc.tile([P, 2 * GPP], f16)
            hpv = hp[:, :].rearrange("p (c g) -> p c g", c=2, g=GPP)
            nc.vector.tensor_mul(
                out=hpv,
                in0=w_sb[:, :].rearrange("p (c g) -> p c g", c=2, g=GPP),
                in1=s1[:, :].rearrange("p g -> p g").unsqueeze(1)
                .broadcast_to((P, 2, GPP)))
            o_sb = acc.tile([P, 2], f32)
            nc.vector.tensor_reduce(
                out=o_sb[:, :].rearrange("p c -> p c"),
                in_=hpv, axis=mybir.AxisListType.X, op=ADD)
            nc.sync.dma_start(out=o_d, in_=o_sb[:, :])

    nc._input_dma_names = {i.ins.name for i in in_dmas}
    return nc


def _get_nc():
    if "nc" not in _cache:
        _cache["nc"] = _build_nc()
    return _cache["nc"]


def _shard(x):
    """(N, ...) f32 -> (CORES, 128, GPP, ...) fp16, zero padded.
    Node mapping: n = (core*128 + p)*GPP + g."""
    out = np.zeros((NP,) + x.shape[1:], np.float16)
    out[:N] = x.astype(np.float16)
    return out.reshape((CORES, P, GPP) + x.shape[1:])


def kernel(h0, cw0, mw0, cw1, mw1,
           msg0_r0, msg0_r1, msg0_r2,
           msg1_r0, msg1_r1, msg1_r2,
           w_pred, b_pred):
    from concourse.bass_utils import run_bass_kernel_spmd

    nc = _get_nc()
    if not _cache.get("split_done"):
        import concourse.mybir as mybir
        _unleash_input_dmas(nc, nc._input_dma_names)
        _split_multiwait(nc, mybir)
        _cache["split_done"] = True

    A = np.asarray(cw0[0], np.float32)
    B = np.asarray(mw0[0], np.float32)
    D = np.asarray(cw1[0], np.float32)
    E = np.asarray(mw1[0], np.float32)

    # ab[n, j, m, k] = {A,B}[n, j, k]; de[n, q, j, i] = {D,E}[n, i, j]
    AB = _shard(np.stack([A, B], axis=2)).reshape(CORES, P, GPP * 512)
    DE = _shard(np.stack([D.transpose(0, 2, 1), E.transpose(0, 2, 1)],
                         axis=1)).reshape(CORES, P, GPP * 512)

    XM = _shard(np.stack([np.asarray(h0, np.float32)[..., 0],
                          np.asarray(msg0_r0, np.float32)[..., 0]],
                         axis=1)).reshape(CORES, P, GPP * 32)
    M1 = _shard(np.asarray(msg1_r0, np.float32)[..., 0]
                ).reshape(CORES, P, GPP * 16)

    wp = np.zeros((2, NP), np.float32)
    wp[:, :N] = np.asarray(w_pred, np.float32)
    W = np.ascontiguousarray(
        wp.reshape(2, CORES, P, GPP).transpose(1, 2, 0, 3)
        .reshape(CORES, P, 2 * GPP)).astype(np.float16)

    in_maps = [
        {"ab": np.ascontiguousarray(AB[i]),
         "de": np.ascontiguousarray(DE[i]),
         "xm": np.ascontiguousarray(XM[i]),
         "m1": np.ascontiguousarray(M1[i]),
         "w": np.ascontiguousarray(W[i])}
        for i in range(CORES)
    ]
    res = run_bass_kernel_spmd(nc, in_maps, list(range(CORES)), trace=TRACE)
    _cache["last_res"] = res
    partial = np.zeros(2, np.float64)
    for i in range(CORES):
        partial += res.results[i]["o"].astype(np.float64).sum(axis=0)
    out = (partial + np.asarray(b_pred, np.float64)).astype(np.float32)
    return out.reshape(1, 2)
